# revision 14
# baseline (speedup 1.0000x reference)
"""Cross-attention kernel for 8 Trainium2 NeuronCores (v5).

Contract: kernel(**inputs) takes FULL unsharded numpy inputs
(x [4,2048,1024], context [4,2048,1024], Wq [1024,1024], Wkv [1024,2048])
and returns the full output [4, 2048, 1024] (float32).

Sharding (hardcoded): core = b * 2 + hg handles batch b (0..3) and head
group hg (0..1) = heads hg*8 .. hg*8+7 (16 heads total, d=64).

v6 over v5 (347us) / v4 (333us):
  - All Schraudolph exps run on the otherwise-idle GPSIMD engine: on the
    DVE they sat in FIFO behind filler casts and finalize ops, delaying
    the exp -> scores PSUM-recycle chain (v5's tail regression).
  - Input DMAs spread over THREE queues (scalar hwdge / sync hwdge /
    gpsimd swdge): per-queue transfer bandwidth is only ~140GB/s, so the
    prefix-critical 1.25MB per consumer lands in ~9us.
  - (GPSIMD cannot read PSUM, so the Schraudolph exps stay on the DVE;
    the v5 tail ACT/DVE half-split is reverted - DVE FIFO queueing made
    it slower than full-width exps.)

Structure (see v2-v4): host-transposed packed inputs (every DMA is 128
descriptors of 1-4KB lines); projections run as deadline-scheduled PE
filler units inside the attention loop; scores use full 128-row
stationaries against zero-padded per-head QTH tiles; exp on ACT for
12/16 j-chunks + one-instruction Schraudolph bf16-bits exp on DVE for
4/16 (imac 0); fused strided-reciprocal + broadcast-multiply finalize;
per-(imac,head) bf16 output blocks reassembled on host.
Per-core PE floor: 592k matmul columns = 246.6us @ 2.4GHz.
"""

import sys

if "/opt/trn_rl_repo" not in sys.path:
    sys.path.insert(0, "/opt/trn_rl_repo")

from collections import defaultdict
from contextlib import ExitStack

import ml_dtypes
import numpy as np

import concourse.bass as bass  # noqa: F401  (registers AP machinery)
import concourse.mybir as mybir
from concourse import bacc
from concourse.bass_utils import run_bass_kernel_spmd
from concourse.tile import TileContext

FP = mybir.dt.float32
BF = mybir.dt.bfloat16
P = 128
SEQ = 2048
DIM = 1024
CC = 512  # per-core channel cols (8 heads x 64)
NH = 8  # heads per core
DH = 64  # head dim
NI = SEQ // P  # 16 j chunks
NK = DIM // P  # 8 contraction chunks
IM = 1024  # i-macro width
NIM = SEQ // IM  # 2
NIC = IM // P  # 8 i-chunks per macro
SCALE = DH ** -0.5  # 1/8
NITER = NIM * NH * NI  # 256 (imac, h, j) iterations
NWARM = 24

A_SCH = float(np.float32(np.log2(np.e) * SCALE * 128.0))
B_SCH = float(np.float32(12582912.0 + 16256.0 - 5.5))
OFF_JS = (2, 6, 10, 14)  # j-chunks whose exp runs on DVE (t >= 16)

EXP = mybir.ActivationFunctionType.Exp
MUL = mybir.AluOpType.mult
ADD = mybir.AluOpType.add

_NC = None


def _build_body(nc, tc, xt_ds, ct_ds, wq_ds, wk_ds, wv_ds, out_d):
    with ExitStack() as ctx:
        wp = ctx.enter_context(tc.tile_pool(name="wp", bufs=8))
        wvp = ctx.enter_context(tc.tile_pool(name="wvp", bufs=2))
        ctsp = ctx.enter_context(tc.tile_pool(name="ctsp", bufs=4))
        xtsp = ctx.enter_context(tc.tile_pool(name="xtsp", bufs=4))
        ktp = ctx.enter_context(tc.tile_pool(name="ktp", bufs=4))
        qtp = ctx.enter_context(tc.tile_pool(name="qtp", bufs=8))
        vp = ctx.enter_context(tc.tile_pool(name="vp", bufs=NI))
        ptp = ctx.enter_context(tc.tile_pool(name="ptp", bufs=6))
        up = ctx.enter_context(tc.tile_pool(name="up", bufs=4))
        outp = ctx.enter_context(tc.tile_pool(name="outp", bufs=4))
        recp = ctx.enter_context(tc.tile_pool(name="recp", bufs=4))
        wrm = ctx.enter_context(tc.tile_pool(name="wrm", bufs=1))
        # PSUM (8 banks): sp 2x2 + at 2x1 + fill 2x1 = 8
        spsum = ctx.enter_context(tc.tile_pool(name="spsum", bufs=2, space="PSUM"))
        apsum = ctx.enter_context(tc.tile_pool(name="apsum", bufs=2, space="PSUM"))
        fillp = ctx.enter_context(tc.tile_pool(name="fillp", bufs=2, space="PSUM"))

        KT = [ktp.tile([P, SEQ], BF, name=f"kt{m}", tag="kt") for m in range(4)]
        QTH = [qtp.tile([P, SEQ], BF, name=f"qth{h}", tag="qth") for h in range(NH)]
        V = [vp.tile([P, NH, DH + 1], BF, name=f"v{j}", tag="v") for j in range(NI)]
        wk = [wp.tile([P, NK, P], BF, name=f"wk{m}", tag="w") for m in range(4)]
        wq = [wp.tile([P, NK, P], BF, name=f"wq{m}", tag="w") for m in range(4)]
        wv = [wvp.tile([P, NK, 256], BF, name=f"wv{hg}", tag="wv") for hg in range(2)]
        cts = [ctsp.tile([P, NK, 512], BF, name=f"ct{q}", tag="cts")
               for q in range(4)]
        xts = [xtsp.tile([P, NK, 512], BF, name=f"xt{q}", tag="xts")
               for q in range(4)]

        # ---- PE warmup: ramp the clock on zeros while DMAs stream in.
        wz = wrm.tile([P, 512], BF, name="wz", tag="wz")
        nc.vector.memset(wz, 0.0)
        wps = fillp.tile([P, 512], FP, name="wps", tag="fp")
        for _ in range(NWARM):
            nc.tensor.matmul(wps, wz[:, 0:P], wz, start=True,
                             stop=True, skip_group_check=True)

        # ---- DMA issue: earliest-needed first; scalar takes the K/V side.
        def dk(eng, tile, dram):
            eng.dma_start(out=tile, in_=dram.rearrange("p (k c) -> p k c", k=NK))

        # critical-before-scores(0): wk0+ct0+xt1 (scalar) and wq0+xt0
        # (sync) stream first; everything else queues behind.  wv rides the
        # gpsimd swdge queue BEHIND its memsets so it does not compete for
        # HBM during the critical window (all queues share ~300GB/s).
        # All input DMAs ride ONE queue (sync hwdge) in priority order:
        # the queues share ~300GB/s of HBM, so a second queue only steals
        # bandwidth from the prefix-critical pieces.  Output DMAs ride
        # scalar.  Order: scores(0) deps, then v(0)/kt(0,1) deps, then bulk.
        dk(nc.sync, wk[0], wk_ds[0])
        dk(nc.sync, cts[0], ct_ds[0])
        dk(nc.sync, wq[0], wq_ds[0])
        dk(nc.sync, xts[0], xt_ds[0])
        dk(nc.sync, xts[1], xt_ds[1])
        dk(nc.sync, wv[0], wv_ds[0])
        dk(nc.sync, cts[1], ct_ds[1])
        dk(nc.sync, wk[1], wk_ds[1])
        dk(nc.sync, wq[1], wq_ds[1])
        dk(nc.sync, wv[1], wv_ds[1])
        dk(nc.sync, xts[2], xt_ds[2])
        dk(nc.sync, xts[3], xt_ds[3])
        dk(nc.sync, cts[2], ct_ds[2])
        dk(nc.sync, cts[3], ct_ds[3])
        dk(nc.sync, wk[2], wk_ds[2])
        dk(nc.sync, wk[3], wk_ds[3])
        dk(nc.sync, wq[2], wq_ds[2])
        dk(nc.sync, wq[3], wq_ds[3])

        # zero halves of QTH (the wrong head's rows)
        for h in range(NH):
            zr = QTH[h][DH:P, :] if h % 2 == 0 else QTH[h][0:DH, :]
            (nc.vector if h < 4 else nc.gpsimd).memset(zr, 0.0)

        # ---- filler units -------------------------------------------------
        def kt_unit(m, i4):
            ps = fillp.tile([P, 512], FP, name="ps", tag="fp")
            for k in range(NK):
                nc.tensor.matmul(ps, wk[m][:, k, :], cts[i4][:, k, :],
                                 start=(k == 0), stop=(k == NK - 1))
            nc.vector.tensor_copy(KT[m][:, i4 * 512:(i4 + 1) * 512], ps)

        def qt_unit(m, i4):
            ps = fillp.tile([P, 512], FP, name="ps", tag="fp")
            for k in range(NK):
                nc.tensor.matmul(ps, wq[m][:, k, :], xts[i4][:, k, :],
                                 start=(k == 0), stop=(k == NK - 1))
            sl = slice(i4 * 512, (i4 + 1) * 512)
            nc.vector.tensor_copy(QTH[2 * m][0:DH, sl], ps[0:DH, :])
            nc.vector.tensor_copy(QTH[2 * m + 1][DH:P, sl], ps[DH:P, :])

        def v_unit(j, hg):
            ps = fillp.tile([P, 256], FP, name="psv", tag="fp")
            for k in range(NK):
                nc.tensor.matmul(
                    ps, cts[j // 4][:, k, (j % 4) * P:(j % 4 + 1) * P],
                    wv[hg][:, k, :], start=(k == 0), stop=(k == NK - 1))
            nc.vector.tensor_copy(
                V[j][:, hg * 4:(hg + 1) * 4, 0:DH],
                ps.rearrange("p (h d) -> p h d", h=4))
            nc.vector.memset(V[j][:, hg * 4:(hg + 1) * 4, DH:DH + 1], 1.0)

        units = []
        for m in range(4):
            for i4 in range(4):
                units.append((32 * m + 4 * i4 - 3, lambda m=m, i4=i4: kt_unit(m, i4)))
                units.append(((i4 // 2) * 128 + 32 * m - 3,
                              lambda m=m, i4=i4: qt_unit(m, i4)))
        for j in range(NI):
            for hg in range(2):
                units.append((64 * hg + j - 2, lambda j=j, hg=hg: v_unit(j, hg)))

        prefix = [fn for dl, fn in sorted(units, key=lambda u: u[0]) if dl < 0]
        main_units = [u for u in units if u[0] >= 0]
        sched = defaultdict(list)
        cursor = 221.0
        step = 221.0 / max(1, len(main_units))
        for dl, fn in sorted(main_units, key=lambda u: -u[0]):
            t = max(0, min(dl, int(cursor)))
            sched[t].append(fn)
            cursor = min(float(dl), cursor) - step

        # ---- attention steady state --------------------------------------
        sps = {}
        pts = {}

        def score_emit(t):
            imac, h, j = t // 128, (t // 16) % 8, t % 16
            m = h // 2
            sp = spsum.tile([P, IM], FP, name="sp", tag="sp")
            for s in range(2):
                nc.tensor.matmul(
                    sp[:, s * 512:(s + 1) * 512],
                    KT[m][:, j * P:(j + 1) * P],
                    QTH[h][:, imac * IM + s * 512:imac * IM + (s + 1) * 512],
                    start=True, stop=True)
            sps[t] = sp

        def exp_emit(t):
            j = t % 16
            sp = sps.pop(t)
            if t >= 16 and j in OFF_JS:
                u = up.tile([P, IM], FP, name="u", tag="u")
                nc.vector.tensor_scalar(u, sp, A_SCH, B_SCH, MUL, ADD)
                pts[t] = ("u", u, None)
            else:
                pt = ptp.tile([P, IM], BF, name="pt", tag="pt")
                nc.scalar.activation(pt, sp, EXP, scale=SCALE)
                pts[t] = ("pt", pt, None)

        def _ubf(u):
            return u.bitcast(BF).rearrange("p (i two) -> p i two", two=2)

        def attn_emit(t, at_g):
            imac, h, j = t // 128, (t // 16) % 8, t % 16
            kind, a, b = pts.pop(t)
            if kind == "u":
                bfv = _ubf(a)
            elif kind == "half":
                bfv = _ubf(b)
            for ic in range(NIC):
                if kind == "u":
                    lhsT = bfv[:, ic * P:(ic + 1) * P, 0]
                elif kind == "half":
                    if ic < 4:
                        lhsT = a[:, ic * P:(ic + 1) * P]
                    else:
                        lhsT = bfv[:, (ic - 4) * P:(ic - 3) * P, 0]
                else:
                    lhsT = a[:, ic * P:(ic + 1) * P]
                nc.tensor.matmul(
                    at_g[ic // 4][:, (ic % 4) * 65:(ic % 4) * 65 + 65],
                    lhsT, V[j][:, h, :],
                    start=(j == 0 and ic % 4 == 0),
                    stop=(j == NI - 1 and ic % 4 == 3),
                    skip_group_check=True)

        def finalize(imac, h, at_g):
            outh = outp.tile([P, 2, 4, DH], BF, name="oh", tag="oh")
            for g in range(2):
                quad = at_g[g][:, 0:260].rearrange("p (q c) -> p q c", c=65)
                rec = recp.tile([P, 4], FP, name="rec", tag="rec")
                nc.vector.reciprocal(rec, quad[:, :, DH])
                nc.vector.tensor_tensor(
                    outh[:, g], quad[:, :, 0:DH],
                    rec.unsqueeze(2).broadcast_to([P, 4, DH]), MUL)
            nc.scalar.dma_start(out=out_d[imac, h],
                               in_=outh.rearrange("p g q c -> p (g q c)"))

        for fn in prefix:
            fn()
        score_emit(0)
        score_emit(1)
        exp_emit(0)

        at_g = None
        for t in range(NITER):
            imac, h, j = t // 128, (t // 16) % 8, t % 16
            if j == 0:
                at_g = [apsum.tile([P, 512], FP, name=f"at{g}", tag="at")
                        for g in range(2)]
            if t + 2 < NITER:
                score_emit(t + 2)
            if t + 1 < NITER:
                exp_emit(t + 1)
            for fn in sched.get(t, ()):
                fn()
            attn_emit(t, at_g)
            if j == NI - 1:
                finalize(imac, h, at_g)


def _build():
    global _NC
    if _NC is not None:
        return _NC
    nc = bacc.Bacc(None, target_bir_lowering=False, debug=False)
    with TileContext(nc) as tc:
        with tc.tile_pool(name="dram", bufs=1, space="DRAM") as dram:
            def din(name, cols, n):
                return [dram.tile([P, NK * cols], BF, kind="ExternalInput",
                                  name=f"{name}{i}", uniquify=False)
                        for i in range(n)]

            xt_ds = din("xt", 512, 4)
            ct_ds = din("ct", 512, 4)
            wq_ds = din("wq", P, 4)
            wk_ds = din("wk", P, 4)
            wv_ds = din("wv", 256, 2)
            out_d = dram.tile([NIM, NH, P, CC], BF, kind="ExternalOutput",
                              name="out", uniquify=False)
            _build_body(nc, tc, xt_ds, ct_ds, wq_ds, wk_ds, wv_ds, out_d)
    nc.compile()
    _NC = nc
    return nc


def _pack_kpc(a):
    # [1024, C] -> [128, 8*C]: row p holds chunk k at cols [k*C, (k+1)*C)
    c = a.shape[1]
    return np.ascontiguousarray(
        a.reshape(NK, P, c).transpose(1, 0, 2).reshape(P, NK * c))


def make_in_maps(x, context, Wq, Wkv):
    bf16 = ml_dtypes.bfloat16
    x = np.asarray(x, dtype=np.float32)
    context = np.asarray(context, dtype=np.float32)
    Wq = np.asarray(Wq, dtype=np.float32).astype(bf16)
    Wkv = np.asarray(Wkv, dtype=np.float32).astype(bf16)
    in_maps = []
    for core in range(8):
        b, hg = divmod(core, 2)
        c0 = hg * CC
        xt = x[b].T.astype(bf16)  # [1024 d, 2048 i]
        ct = context[b].T.astype(bf16)
        wqs = Wq[:, c0:c0 + CC]
        wks = Wkv[:, c0:c0 + CC]
        wvs = Wkv[:, DIM + c0:DIM + c0 + CC]
        m = {}
        for q in range(4):
            m[f"xt{q}"] = _pack_kpc(xt[:, q * 512:(q + 1) * 512])
            m[f"ct{q}"] = _pack_kpc(ct[:, q * 512:(q + 1) * 512])
            m[f"wq{q}"] = _pack_kpc(wqs[:, q * P:(q + 1) * P])
            m[f"wk{q}"] = _pack_kpc(wks[:, q * P:(q + 1) * P])
        for hg2 in range(2):
            m[f"wv{hg2}"] = _pack_kpc(wvs[:, hg2 * 256:(hg2 + 1) * 256])
        in_maps.append(m)
    return in_maps


def run(x, context, Wq, Wkv, **run_kwargs):
    nc = _build()
    in_maps = make_in_maps(x, context, Wq, Wkv)
    res = run_bass_kernel_spmd(nc, in_maps, core_ids=list(range(8)), **run_kwargs)
    out = np.empty((4, SEQ, DIM), dtype=np.float32)
    for core in range(8):
        b, hg = divmod(core, 2)
        r = np.asarray(res.results[core]["out"]).astype(np.float32)
        r = r.reshape(NIM, NH, P, NIC, DH).transpose(0, 3, 2, 1, 4)
        out[b, :, hg * CC:(hg + 1) * CC] = r.reshape(SEQ, CC)
    return out, res


def kernel(x, context, Wq, Wkv):
    out, _ = run(x, context, Wq, Wkv)
    return out


# revision 16
# speedup vs baseline: 1.0164x; 1.0164x over previous
"""Cross-attention kernel for 8 Trainium2 NeuronCores (v5).

Contract: kernel(**inputs) takes FULL unsharded numpy inputs
(x [4,2048,1024], context [4,2048,1024], Wq [1024,1024], Wkv [1024,2048])
and returns the full output [4, 2048, 1024] (float32).

Sharding (hardcoded): core = b * 2 + hg handles batch b (0..3) and head
group hg (0..1) = heads hg*8 .. hg*8+7 (16 heads total, d=64).

v6 over v5 (347us) / v4 (333us):
  - All Schraudolph exps run on the otherwise-idle GPSIMD engine: on the
    DVE they sat in FIFO behind filler casts and finalize ops, delaying
    the exp -> scores PSUM-recycle chain (v5's tail regression).
  - Input DMAs spread over THREE queues (scalar hwdge / sync hwdge /
    gpsimd swdge): per-queue transfer bandwidth is only ~140GB/s, so the
    prefix-critical 1.25MB per consumer lands in ~9us.
  - (GPSIMD cannot read PSUM, so the Schraudolph exps stay on the DVE;
    the v5 tail ACT/DVE half-split is reverted - DVE FIFO queueing made
    it slower than full-width exps.)

Structure (see v2-v4): host-transposed packed inputs (every DMA is 128
descriptors of 1-4KB lines); projections run as deadline-scheduled PE
filler units inside the attention loop; scores use full 128-row
stationaries against zero-padded per-head QTH tiles; exp on ACT for
12/16 j-chunks + one-instruction Schraudolph bf16-bits exp on DVE for
4/16 (imac 0); fused strided-reciprocal + broadcast-multiply finalize;
per-(imac,head) bf16 output blocks reassembled on host.
Per-core PE floor: 592k matmul columns = 246.6us @ 2.4GHz.
"""

import sys

if "/opt/trn_rl_repo" not in sys.path:
    sys.path.insert(0, "/opt/trn_rl_repo")

from collections import defaultdict
from contextlib import ExitStack

import ml_dtypes
import numpy as np

import concourse.bass as bass  # noqa: F401  (registers AP machinery)
import concourse.mybir as mybir
from concourse import bacc
from concourse.bass_utils import run_bass_kernel_spmd
from concourse.tile import TileContext

FP = mybir.dt.float32
BF = mybir.dt.bfloat16
P = 128
SEQ = 2048
DIM = 1024
CC = 512  # per-core channel cols (8 heads x 64)
NH = 8  # heads per core
DH = 64  # head dim
NI = SEQ // P  # 16 j chunks
NK = DIM // P  # 8 contraction chunks
IM = 1024  # i-macro width
NIM = SEQ // IM  # 2
NIC = IM // P  # 8 i-chunks per macro
SCALE = DH ** -0.5  # 1/8
NITER = NIM * NH * NI  # 256 (imac, h, j) iterations
NWARM = 24

A_SCH = float(np.float32(np.log2(np.e) * SCALE * 128.0))
B_SCH = float(np.float32(12582912.0 + 16256.0 - 5.5))
OFF_JS = (2, 6, 10, 14)  # j-chunks whose exp runs on DVE (t >= 16)

EXP = mybir.ActivationFunctionType.Exp
MUL = mybir.AluOpType.mult
ADD = mybir.AluOpType.add

_NC = None


def _build_body(nc, tc, xt_ds, ct_ds, wq_ds, wk_ds, wv_ds, out_d):
    with ExitStack() as ctx:
        wp = ctx.enter_context(tc.tile_pool(name="wp", bufs=8))
        wvp = ctx.enter_context(tc.tile_pool(name="wvp", bufs=2))
        ctsp = ctx.enter_context(tc.tile_pool(name="ctsp", bufs=4))
        xtsp = ctx.enter_context(tc.tile_pool(name="xtsp", bufs=4))
        ktp = ctx.enter_context(tc.tile_pool(name="ktp", bufs=4))
        qtp = ctx.enter_context(tc.tile_pool(name="qtp", bufs=8))
        vp = ctx.enter_context(tc.tile_pool(name="vp", bufs=NI))
        ptp = ctx.enter_context(tc.tile_pool(name="ptp", bufs=6))
        up = ctx.enter_context(tc.tile_pool(name="up", bufs=4))
        outp = ctx.enter_context(tc.tile_pool(name="outp", bufs=4))
        recp = ctx.enter_context(tc.tile_pool(name="recp", bufs=4))
        wrm = ctx.enter_context(tc.tile_pool(name="wrm", bufs=1))
        # PSUM (8 banks): sp 2x2 + at 2x1 + fill 2x1 = 8
        spsum = ctx.enter_context(tc.tile_pool(name="spsum", bufs=2, space="PSUM"))
        apsum = ctx.enter_context(tc.tile_pool(name="apsum", bufs=2, space="PSUM"))
        fillp = ctx.enter_context(tc.tile_pool(name="fillp", bufs=2, space="PSUM"))

        KT = [ktp.tile([P, SEQ], BF, name=f"kt{m}", tag="kt") for m in range(4)]
        QTH = [qtp.tile([P, SEQ], BF, name=f"qth{h}", tag="qth") for h in range(NH)]
        V = [vp.tile([P, NH, DH + 1], BF, name=f"v{j}", tag="v") for j in range(NI)]
        wk = [wp.tile([P, NK, P], BF, name=f"wk{m}", tag="w") for m in range(4)]
        wq = [wp.tile([P, NK, P], BF, name=f"wq{m}", tag="w") for m in range(4)]
        wv = [wvp.tile([P, NK, 256], BF, name=f"wv{hg}", tag="wv") for hg in range(2)]
        cts = [ctsp.tile([P, NK, 512], BF, name=f"ct{q}", tag="cts")
               for q in range(4)]
        xts = [xtsp.tile([P, NK, 512], BF, name=f"xt{q}", tag="xts")
               for q in range(4)]

        # ---- PE warmup: ramp the clock on zeros while DMAs stream in.
        wz = wrm.tile([P, 512], BF, name="wz", tag="wz")
        nc.vector.memset(wz, 0.0)
        wps = fillp.tile([P, 512], FP, name="wps", tag="fp")
        for _ in range(NWARM):
            nc.tensor.matmul(wps, wz[:, 0:P], wz, start=True,
                             stop=True, skip_group_check=True)

        # ---- DMA issue: earliest-needed first; scalar takes the K/V side.
        def dk(eng, tile, dram):
            eng.dma_start(out=tile, in_=dram.rearrange("p (k c) -> p k c", k=NK))

        # critical-before-scores(0): wk0+ct0+xt1 (scalar) and wq0+xt0
        # (sync) stream first; everything else queues behind.  wv rides the
        # gpsimd swdge queue BEHIND its memsets so it does not compete for
        # HBM during the critical window (all queues share ~300GB/s).
        # Prefix-critical pieces (scores(0) deps) ride the sync hwdge
        # queue first; bulk rides scalar so the critical ~4MB is not stuck
        # behind it (queues share ~300GB/s of HBM).
        dk(nc.sync, wk[0], wk_ds[0])
        dk(nc.sync, cts[0], ct_ds[0])
        dk(nc.sync, wq[0], wq_ds[0])
        dk(nc.sync, xts[0], xt_ds[0])
        dk(nc.sync, xts[1], xt_ds[1])
        dk(nc.sync, wv[0], wv_ds[0])
        dk(nc.scalar, cts[1], ct_ds[1])
        dk(nc.scalar, wk[1], wk_ds[1])
        dk(nc.scalar, wq[1], wq_ds[1])
        dk(nc.scalar, wv[1], wv_ds[1])
        dk(nc.scalar, xts[2], xt_ds[2])
        dk(nc.scalar, xts[3], xt_ds[3])
        dk(nc.scalar, cts[2], ct_ds[2])
        dk(nc.scalar, cts[3], ct_ds[3])
        dk(nc.scalar, wk[2], wk_ds[2])
        dk(nc.scalar, wk[3], wk_ds[3])
        dk(nc.scalar, wq[2], wq_ds[2])
        dk(nc.scalar, wq[3], wq_ds[3])

        # zero halves of QTH (the wrong head's rows)
        for h in range(NH):
            zr = QTH[h][DH:P, :] if h % 2 == 0 else QTH[h][0:DH, :]
            (nc.vector if h < 4 else nc.gpsimd).memset(zr, 0.0)

        # ---- filler units -------------------------------------------------
        def kt_unit(m, i4):
            ps = fillp.tile([P, 512], FP, name="ps", tag="fp")
            for k in range(NK):
                nc.tensor.matmul(ps, wk[m][:, k, :], cts[i4][:, k, :],
                                 start=(k == 0), stop=(k == NK - 1))
            nc.vector.tensor_copy(KT[m][:, i4 * 512:(i4 + 1) * 512], ps)

        def qt_unit(m, i4):
            ps = fillp.tile([P, 512], FP, name="ps", tag="fp")
            for k in range(NK):
                nc.tensor.matmul(ps, wq[m][:, k, :], xts[i4][:, k, :],
                                 start=(k == 0), stop=(k == NK - 1))
            sl = slice(i4 * 512, (i4 + 1) * 512)
            nc.vector.tensor_copy(QTH[2 * m][0:DH, sl], ps[0:DH, :])
            nc.vector.tensor_copy(QTH[2 * m + 1][DH:P, sl], ps[DH:P, :])

        def v_unit(j, hg):
            ps = fillp.tile([P, 256], FP, name="psv", tag="fp")
            for k in range(NK):
                nc.tensor.matmul(
                    ps, cts[j // 4][:, k, (j % 4) * P:(j % 4 + 1) * P],
                    wv[hg][:, k, :], start=(k == 0), stop=(k == NK - 1))
            nc.vector.tensor_copy(
                V[j][:, hg * 4:(hg + 1) * 4, 0:DH],
                ps.rearrange("p (h d) -> p h d", h=4))
            nc.vector.memset(V[j][:, hg * 4:(hg + 1) * 4, DH:DH + 1], 1.0)

        units = []
        for m in range(4):
            for i4 in range(4):
                units.append((32 * m + 4 * i4 - 3, lambda m=m, i4=i4: kt_unit(m, i4)))
                units.append(((i4 // 2) * 128 + 32 * m - 3,
                              lambda m=m, i4=i4: qt_unit(m, i4)))
        for j in range(NI):
            for hg in range(2):
                units.append((64 * hg + j - 2, lambda j=j, hg=hg: v_unit(j, hg)))

        prefix = [fn for dl, fn in sorted(units, key=lambda u: u[0]) if dl < 0]
        main_units = [u for u in units if u[0] >= 0]
        sched = defaultdict(list)
        cursor = 221.0
        step = 221.0 / max(1, len(main_units))
        for dl, fn in sorted(main_units, key=lambda u: -u[0]):
            t = max(0, min(dl, int(cursor)))
            sched[t].append(fn)
            cursor = min(float(dl), cursor) - step

        # ---- attention steady state --------------------------------------
        sps = {}
        pts = {}

        def score_emit(t):
            imac, h, j = t // 128, (t // 16) % 8, t % 16
            m = h // 2
            sp = spsum.tile([P, IM], FP, name="sp", tag="sp")
            for s in range(2):
                nc.tensor.matmul(
                    sp[:, s * 512:(s + 1) * 512],
                    KT[m][:, j * P:(j + 1) * P],
                    QTH[h][:, imac * IM + s * 512:imac * IM + (s + 1) * 512],
                    start=True, stop=True)
            sps[t] = sp

        def exp_emit(t):
            j = t % 16
            sp = sps.pop(t)
            if t >= 16 and j in OFF_JS:
                u = up.tile([P, IM], FP, name="u", tag="u")
                nc.vector.tensor_scalar(u, sp, A_SCH, B_SCH, MUL, ADD)
                pts[t] = ("u", u, None)
            else:
                pt = ptp.tile([P, IM], BF, name="pt", tag="pt")
                nc.scalar.activation(pt, sp, EXP, scale=SCALE)
                pts[t] = ("pt", pt, None)

        def _ubf(u):
            return u.bitcast(BF).rearrange("p (i two) -> p i two", two=2)

        def attn_emit(t, at_g):
            imac, h, j = t // 128, (t // 16) % 8, t % 16
            kind, a, b = pts.pop(t)
            if kind == "u":
                bfv = _ubf(a)
            elif kind == "half":
                bfv = _ubf(b)
            for ic in range(NIC):
                if kind == "u":
                    lhsT = bfv[:, ic * P:(ic + 1) * P, 0]
                elif kind == "half":
                    if ic < 4:
                        lhsT = a[:, ic * P:(ic + 1) * P]
                    else:
                        lhsT = bfv[:, (ic - 4) * P:(ic - 3) * P, 0]
                else:
                    lhsT = a[:, ic * P:(ic + 1) * P]
                nc.tensor.matmul(
                    at_g[ic // 4][:, (ic % 4) * 65:(ic % 4) * 65 + 65],
                    lhsT, V[j][:, h, :],
                    start=(j == 0 and ic % 4 == 0),
                    stop=(j == NI - 1 and ic % 4 == 3),
                    skip_group_check=True)

        def finalize(imac, h, at_g):
            outh = outp.tile([P, 2, 4, DH], BF, name="oh", tag="oh")
            for g in range(2):
                quad = at_g[g][:, 0:260].rearrange("p (q c) -> p q c", c=65)
                rec = recp.tile([P, 4], FP, name="rec", tag="rec")
                nc.vector.reciprocal(rec, quad[:, :, DH])
                nc.vector.tensor_tensor(
                    outh[:, g], quad[:, :, 0:DH],
                    rec.unsqueeze(2).broadcast_to([P, 4, DH]), MUL)
            eng = nc.sync if h % 2 == 0 else nc.scalar
            eng.dma_start(out=out_d[imac, h],
                          in_=outh.rearrange("p g q c -> p (g q c)"))

        def warm(n):
            w = fillp.tile([P, 512], FP, name="wps2", tag="fp")
            for _ in range(n):
                nc.tensor.matmul(w, wz[:, 0:P], wz, start=True,
                                 stop=True, skip_group_check=True)

        for fn in prefix:
            fn()
            warm(8)
        score_emit(0)
        score_emit(1)
        exp_emit(0)

        at_g = None
        for t in range(NITER):
            imac, h, j = t // 128, (t // 16) % 8, t % 16
            if j == 0:
                at_g = [apsum.tile([P, 512], FP, name=f"at{g}", tag="at")
                        for g in range(2)]
            if t + 2 < NITER:
                score_emit(t + 2)
            if t + 1 < NITER:
                exp_emit(t + 1)
            for fn in sched.get(t, ()):
                fn()
            attn_emit(t, at_g)
            if j == NI - 1:
                finalize(imac, h, at_g)


def _build():
    global _NC
    if _NC is not None:
        return _NC
    nc = bacc.Bacc(None, target_bir_lowering=False, debug=False)
    with TileContext(nc) as tc:
        with tc.tile_pool(name="dram", bufs=1, space="DRAM") as dram:
            def din(name, cols, n):
                return [dram.tile([P, NK * cols], BF, kind="ExternalInput",
                                  name=f"{name}{i}", uniquify=False)
                        for i in range(n)]

            xt_ds = din("xt", 512, 4)
            ct_ds = din("ct", 512, 4)
            wq_ds = din("wq", P, 4)
            wk_ds = din("wk", P, 4)
            wv_ds = din("wv", 256, 2)
            out_d = dram.tile([NIM, NH, P, CC], BF, kind="ExternalOutput",
                              name="out", uniquify=False)
            _build_body(nc, tc, xt_ds, ct_ds, wq_ds, wk_ds, wv_ds, out_d)
    nc.compile()
    _NC = nc
    return nc


def _pack_kpc(a):
    # [1024, C] -> [128, 8*C]: row p holds chunk k at cols [k*C, (k+1)*C)
    c = a.shape[1]
    return np.ascontiguousarray(
        a.reshape(NK, P, c).transpose(1, 0, 2).reshape(P, NK * c))


def make_in_maps(x, context, Wq, Wkv):
    bf16 = ml_dtypes.bfloat16
    x = np.asarray(x, dtype=np.float32)
    context = np.asarray(context, dtype=np.float32)
    Wq = np.asarray(Wq, dtype=np.float32).astype(bf16)
    Wkv = np.asarray(Wkv, dtype=np.float32).astype(bf16)
    in_maps = []
    for core in range(8):
        b, hg = divmod(core, 2)
        c0 = hg * CC
        xt = x[b].T.astype(bf16)  # [1024 d, 2048 i]
        ct = context[b].T.astype(bf16)
        wqs = Wq[:, c0:c0 + CC]
        wks = Wkv[:, c0:c0 + CC]
        wvs = Wkv[:, DIM + c0:DIM + c0 + CC]
        m = {}
        for q in range(4):
            m[f"xt{q}"] = _pack_kpc(xt[:, q * 512:(q + 1) * 512])
            m[f"ct{q}"] = _pack_kpc(ct[:, q * 512:(q + 1) * 512])
            m[f"wq{q}"] = _pack_kpc(wqs[:, q * P:(q + 1) * P])
            m[f"wk{q}"] = _pack_kpc(wks[:, q * P:(q + 1) * P])
        for hg2 in range(2):
            m[f"wv{hg2}"] = _pack_kpc(wvs[:, hg2 * 256:(hg2 + 1) * 256])
        in_maps.append(m)
    return in_maps


def run(x, context, Wq, Wkv, **run_kwargs):
    nc = _build()
    in_maps = make_in_maps(x, context, Wq, Wkv)
    res = run_bass_kernel_spmd(nc, in_maps, core_ids=list(range(8)), **run_kwargs)
    out = np.empty((4, SEQ, DIM), dtype=np.float32)
    for core in range(8):
        b, hg = divmod(core, 2)
        r = np.asarray(res.results[core]["out"]).astype(np.float32)
        r = r.reshape(NIM, NH, P, NIC, DH).transpose(0, 3, 2, 1, 4)
        out[b, :, hg * CC:(hg + 1) * CC] = r.reshape(SEQ, CC)
    return out, res


def kernel(x, context, Wq, Wkv):
    out, _ = run(x, context, Wq, Wkv)
    return out
